# revision 3
# baseline (speedup 1.0000x reference)
"""Trainium2 Bass kernel for the HNN sparse-MLP network.

Strategy: the sparse layers have fixed connectivity, so we densify the
sparse weight lists into dense matrices on the host and run the whole
network as dense fp32r matmuls on the tensor engine, data-parallel over
the batch across 8 NeuronCores (1024 rows each).

Layout: activations live feature-on-partition ([features, batch]) the
whole way through, so no transposes are needed between layers:
    h_out[f_out, b] = relu( sum_k W[f_in, f_out]^T . h_in[f_in, b] + bias )
with lhsT = W k-tile [128, Mw], rhs = h_in k-tile [128, 512].

The scalar fc taps (fc1..fc4) are folded in as one extra output feature
per layer (an Mw=1 matmul tile); the final readout is a K=4 matmul over
the concatenated taps.

fp32r (fp32 rounded to 11-bit mantissa) runs the PE at full rate
(1 col/cycle, 4x faster than plain fp32) at ~1e-4 relative error.
Inputs are pre-rounded on the host so DMAs can feed fp32r tiles
directly.
"""

import sys

sys.path.insert(0, "/opt/trn_rl_repo")

import numpy as np

import concourse.bass as bass
import concourse.tile as tile
import concourse.mybir as mybir
from concourse import bacc, bass_utils

F32 = mybir.dt.float32
F32R = mybir.dt.float32r
RELU = mybir.ActivationFunctionType.Relu
COPY = mybir.ActivationFunctionType.Copy

NCORES = 8
B, L1, L2, L3, L4 = 8192, 4096, 2048, 1024, 512
BC = B // NCORES          # batch rows per core
NB = 512                  # matmul moving free dim (PSUM bank limit for fp32)
NBLK = BC // NB           # N-blocks per core


def round_fp32r(a: np.ndarray) -> np.ndarray:
    """Round fp32 to fp32r (11-bit mantissa, RNE) = walrus fp32_to_fp32r."""
    u = np.ascontiguousarray(a, dtype=np.float32).view(np.uint32)
    lsb = (u >> 12) & 1
    r = (u + 0x7FF + lsb) & np.uint32(0xFFFFF000)
    return r.view(np.float32)


def _densify(w, out_idx, in_idx, fc_w, in_dim, out_dim):
    """Dense [in_dim, out_dim+1] matrix from edge lists + fc column."""
    wd = np.zeros((in_dim, out_dim + 1), np.float32)
    np.add.at(wd, (np.asarray(in_idx), np.asarray(out_idx)), np.asarray(w, np.float32))
    wd[:, out_dim] = np.asarray(fc_w, np.float32).reshape(-1)
    return wd


def _pack_w(wd, in_dim, out_dim):
    """Pack dense [in_dim, out_dim+1] into per-M-block contiguous tiles.

    Returns (wp [T, 128, K/128*128], wfc [128, K/128], T) where
    wp[t, p, j*128+m] = wd[j*128+p, t*128+m] and wfc[p, j] = wd[j*128+p, out_dim].
    """
    kt = in_dim // 128
    t = out_dim // 128
    wmain = wd[:, :out_dim].reshape(kt, 128, t, 128)
    wp = np.ascontiguousarray(wmain.transpose(2, 1, 0, 3).reshape(t, 128, kt * 128))
    wfc = np.ascontiguousarray(wd[:, out_dim].reshape(kt, 128).T)
    return round_fp32r(wp), round_fp32r(wfc), t


def _pack_b(b, fc_b, out_dim):
    """Pack bias [out_dim] + fc bias into [128, T+1] (column t = tile t)."""
    t = out_dim // 128
    bp = np.zeros((128, t + 1), np.float32)
    bp[:, :t] = np.asarray(b, np.float32).reshape(t, 128).T
    bp[0, t] = float(np.asarray(fc_b).reshape(-1)[0])
    return bp


def _build_program():
    nc = bacc.Bacc("TRN2", target_bir_lowering=False, debug=False,
                   num_devices=NCORES)
    d = {}
    d["xt"] = nc.dram_tensor("xt", [L1, BC], F32R, kind="ExternalInput").ap()
    d["w1p"] = nc.dram_tensor("w1p", [16, 128, L1], F32R, kind="ExternalInput").ap()
    d["w1fc"] = nc.dram_tensor("w1fc", [128, 32], F32R, kind="ExternalInput").ap()
    d["b1"] = nc.dram_tensor("b1", [128, 17], F32, kind="ExternalInput").ap()
    d["w2p"] = nc.dram_tensor("w2p", [8, 128, L2], F32R, kind="ExternalInput").ap()
    d["w2fc"] = nc.dram_tensor("w2fc", [128, 16], F32R, kind="ExternalInput").ap()
    d["b2"] = nc.dram_tensor("b2", [128, 9], F32, kind="ExternalInput").ap()
    d["w3p"] = nc.dram_tensor("w3p", [4, 128, L3], F32R, kind="ExternalInput").ap()
    d["w3fc"] = nc.dram_tensor("w3fc", [128, 8], F32R, kind="ExternalInput").ap()
    d["b3"] = nc.dram_tensor("b3", [128, 5], F32, kind="ExternalInput").ap()
    d["w4"] = nc.dram_tensor("w4", [128, 4], F32R, kind="ExternalInput").ap()
    d["fc4b"] = nc.dram_tensor("fc4b", [1, 1], F32, kind="ExternalInput").ap()
    d["rw"] = nc.dram_tensor("rw", [4, 1], F32R, kind="ExternalInput").ap()
    d["rb"] = nc.dram_tensor("rb", [1, 1], F32, kind="ExternalInput").ap()
    out_d = nc.dram_tensor("out", [1, BC], F32, kind="ExternalOutput").ap()

    with tile.TileContext(nc) as tc:
        _emit(nc, tc, d, out_d)
    nc.compile()
    return nc


def _emit(nc, tc, d, out_d):
    from contextlib import ExitStack

    with ExitStack() as ctx:
        consts = ctx.enter_context(tc.tile_pool(name="consts", bufs=1))
        psum = ctx.enter_context(tc.tile_pool(name="psum", bufs=4, space="PSUM"))
        stage = ctx.enter_context(tc.tile_pool(name="stage", bufs=4))
        dram = ctx.enter_context(tc.tile_pool(name="dram", bufs=1, space="DRAM"))

        def cload(name, shape, dt):
            t = consts.tile(shape, dt, tag=name)
            nc.sync.dma_start(t[:], d[name][:])
            return t

        b1sb = cload("b1", [128, 17], F32)
        b2sb = cload("b2", [128, 9], F32)
        b3sb = cload("b3", [128, 5], F32)
        w1fc = cload("w1fc", [128, 32], F32R)
        w2fc = cload("w2fc", [128, 16], F32R)
        w3fc = cload("w3fc", [128, 8], F32R)
        w4sb = cload("w4", [128, 4], F32R)
        fc4b = cload("fc4b", [1, 1], F32)
        rwsb = cload("rw", [4, 1], F32R)
        rbsb = cload("rb", [1, 1], F32)

        h1d = dram.tile([17 * 128, BC], F32R)

        # ---- layer 1: x [4096, BC] -> h1 [2049, BC] (spilled to DRAM) ----
        with tc.tile_pool(name="xts", bufs=32) as xpool, \
             tc.tile_pool(name="w1m", bufs=2) as w1pool:
            xts = []
            xview = d["xt"].rearrange("(j p) b -> p j b", p=128)
            for j in range(32):
                xt = xpool.tile([128, BC], F32R, tag="xts")
                nc.sync.dma_start(xt[:], xview[:, j, :])
                xts.append(xt)

            for m in range(17):
                if m < 16:
                    mw = 128
                    wm = w1pool.tile([128, 32 * 128], F32R, tag="w1m")
                    nc.sync.dma_start(wm[:], d["w1p"][m])
                else:
                    mw = 1
                    wm = w1fc
                for nb in range(NBLK):
                    pt = psum.tile([128, NB], F32)
                    for k in range(32):
                        nc.tensor.matmul(
                            pt[:mw], wm[:, k * mw:(k + 1) * mw],
                            xts[k][:, nb * NB:(nb + 1) * NB],
                            start=(k == 0), stop=(k == 31))
                    st = stage.tile([128, NB], F32R, tag="stage")
                    nc.scalar.activation(st[:mw], pt[:mw], RELU,
                                         bias=b1sb[:mw, m:m + 1])
                    nc.sync.dma_start(
                        h1d[m * 128:m * 128 + mw, nb * NB:(nb + 1) * NB],
                        st[:mw])

        # ---- layer 2: h1 [2048, BC] -> h2 [1025, BC] (SBUF-resident) ----
        h2pool = ctx.enter_context(tc.tile_pool(name="h2", bufs=9))
        h2ts = [h2pool.tile([128, BC], F32R, tag="h2", name=f"h2_{i}") for i in range(9)]
        with tc.tile_pool(name="h1ts", bufs=16) as h1pool, \
             tc.tile_pool(name="w2m", bufs=2) as w2pool:
            h1ts = []
            for j in range(16):
                t = h1pool.tile([128, BC], F32R, tag="h1ts")
                nc.sync.dma_start(t[:], h1d[j * 128:(j + 1) * 128, :])
                h1ts.append(t)

            for m in range(9):
                if m < 8:
                    mw = 128
                    wm = w2pool.tile([128, 16 * 128], F32R, tag="w2m")
                    nc.sync.dma_start(wm[:], d["w2p"][m])
                else:
                    mw = 1
                    wm = w2fc
                for nb in range(NBLK):
                    pt = psum.tile([128, NB], F32)
                    for k in range(16):
                        nc.tensor.matmul(
                            pt[:mw], wm[:, k * mw:(k + 1) * mw],
                            h1ts[k][:, nb * NB:(nb + 1) * NB],
                            start=(k == 0), stop=(k == 15))
                    nc.scalar.activation(
                        h2ts[m][:mw, nb * NB:(nb + 1) * NB], pt[:mw], RELU,
                        bias=b2sb[:mw, m:m + 1])

        # ---- layer 3: h2 [1024, BC] -> h3 [513, BC] ----
        h3pool = ctx.enter_context(tc.tile_pool(name="h3", bufs=5))
        h3ts = [h3pool.tile([128, BC], F32R, tag="h3", name=f"h3_{i}") for i in range(5)]
        with tc.tile_pool(name="w3m", bufs=2) as w3pool:
            for m in range(5):
                if m < 4:
                    mw = 128
                    wm = w3pool.tile([128, 8 * 128], F32R, tag="w3m")
                    nc.sync.dma_start(wm[:], d["w3p"][m])
                else:
                    mw = 1
                    wm = w3fc
                for nb in range(NBLK):
                    pt = psum.tile([128, NB], F32)
                    for k in range(8):
                        nc.tensor.matmul(
                            pt[:mw], wm[:, k * mw:(k + 1) * mw],
                            h2ts[k][:, nb * NB:(nb + 1) * NB],
                            start=(k == 0), stop=(k == 7))
                    nc.scalar.activation(
                        h3ts[m][:mw, nb * NB:(nb + 1) * NB], pt[:mw], RELU,
                        bias=b3sb[:mw, m:m + 1])

        # ---- fc4 tap: h3 [512, BC] -> f4 [1, BC] ----
        f4sb = consts.tile([1, BC], F32R, tag="f4")
        for nb in range(NBLK):
            pt = psum.tile([128, NB], F32)
            for k in range(4):
                nc.tensor.matmul(pt[:1], w4sb[:, k:k + 1],
                                 h3ts[k][:, nb * NB:(nb + 1) * NB],
                                 start=(k == 0), stop=(k == 3))
            nc.scalar.activation(f4sb[:1, nb * NB:(nb + 1) * NB], pt[:1],
                                 RELU, bias=fc4b[:1])

        # ---- readout: out = ro_w . [f1 f2 f3 f4] + ro_b ----
        cat = consts.tile([4, BC], F32R, tag="cat")
        nc.sync.dma_start(cat[0:1, :], h1d[16 * 128:16 * 128 + 1, :])
        nc.sync.dma_start(cat[1:2, :], h2ts[8][0:1, :])
        nc.sync.dma_start(cat[2:3, :], h3ts[4][0:1, :])
        nc.sync.dma_start(cat[3:4, :], f4sb[0:1, :])
        outsb = consts.tile([1, BC], F32, tag="outsb")
        for nb in range(NBLK):
            pt = psum.tile([128, NB], F32)
            nc.tensor.matmul(pt[:1], rwsb[:], cat[:, nb * NB:(nb + 1) * NB],
                             start=True, stop=True)
            nc.vector.tensor_scalar_add(outsb[:1, nb * NB:(nb + 1) * NB],
                                        pt[:1], rbsb[:1])
        nc.sync.dma_start(out_d[:], outsb[:1, :])


_NC_CACHE = None


def _get_program():
    global _NC_CACHE
    if _NC_CACHE is None:
        _NC_CACHE = _build_program()
    return _NC_CACHE


def _prepare_in_maps(inputs):
    x = np.asarray(inputs["x"], np.float32)
    w1d = _densify(inputs["sl1_w"], inputs["sl1_out"], inputs["sl1_in"],
                   inputs["fc1_w"], L1, L2)
    w2d = _densify(inputs["sl2_w"], inputs["sl2_out"], inputs["sl2_in"],
                   inputs["fc2_w"], L2, L3)
    w3d = _densify(inputs["sl3_w"], inputs["sl3_out"], inputs["sl3_in"],
                   inputs["fc3_w"], L3, L4)
    w1p, w1fc, _ = _pack_w(w1d, L1, L2)
    w2p, w2fc, _ = _pack_w(w2d, L2, L3)
    w3p, w3fc, _ = _pack_w(w3d, L3, L4)
    shared = {
        "w1p": w1p, "w1fc": w1fc,
        "b1": _pack_b(inputs["sl1_b"], inputs["fc1_b"], L2),
        "w2p": w2p, "w2fc": w2fc,
        "b2": _pack_b(inputs["sl2_b"], inputs["fc2_b"], L3),
        "w3p": w3p, "w3fc": w3fc,
        "b3": _pack_b(inputs["sl3_b"], inputs["fc3_b"], L4),
        "w4": round_fp32r(np.asarray(inputs["fc4_w"], np.float32)
                          .reshape(4, 128).T.copy()),
        "fc4b": np.asarray(inputs["fc4_b"], np.float32).reshape(1, 1),
        "rw": round_fp32r(np.asarray(inputs["ro_w"], np.float32)
                          .reshape(4, 1).copy()),
        "rb": np.asarray(inputs["ro_b"], np.float32).reshape(1, 1),
    }
    in_maps = []
    for c in range(NCORES):
        xt = round_fp32r(
            np.ascontiguousarray(x[c * BC:(c + 1) * BC, :].T))
        in_maps.append({"xt": xt, **shared})
    return in_maps


def run(inputs, **kw):
    nc = _get_program()
    in_maps = _prepare_in_maps(inputs)
    res = bass_utils.run_bass_kernel_spmd(
        nc, in_maps, core_ids=list(range(NCORES)), **kw)
    out = np.concatenate([res.results[c]["out"].reshape(BC)
                          for c in range(NCORES)])
    return out.reshape(B, 1), res


def kernel(**inputs) -> np.ndarray:
    out, _ = run(inputs)
    return out
